# revision 5
# baseline (speedup 1.0000x reference)
"""Compact Bilinear Pooling on 8 Trainium2 NeuronCores (Bass/Tile).

Math: the reference computes, per batch image b,
    out[b] = sum_{pixels n} irfft( rfft(b1[n] @ S1) * rfft(b2[n] @ S2) )
Since irfft is linear and the sum-pool happens after it, this collapses to
    out[b] = irfft( sum_n rfft(sketch1[n]) * rfft(sketch2[n]) ).
And rfft(x @ S) = x @ F where F[c, k] = s[c] * exp(-2i*pi*h[c]*k/D) is a dense
"DFT of the count-sketch" matrix, precomputable on the host from S (each row of
S has a single nonzero s[c] at column h[c]).  So per batch image:
  stage 1: fftX[k, n] = sum_c F[c, k] * bX[c, n]      (PE matmuls, bf16)
  stage 2: Spec[k]    = sum_n fft1[k, n]*fft2[k, n]   (fused DVE mult+reduce)
  stage 4: out[b]     = irfft(Spec)                   (tiny Cooley-Tukey 128x64
                                                       via fp32 PE matmuls)
Conjugate symmetry: only k = 0..4096 computed (padded to 33 tiles of 128).
DC/Nyquist 1/2-weights and the zeroing of pad bins are folded into F1 on host.
Sharding: data-parallel over batch, 4 images per core, no cross-core comm.
"""

import numpy as np
import ml_dtypes

import concourse.bass as bass
import concourse.bacc as bacc
import concourse.mybir as mybir
import concourse.tile as tile
from concourse.bass_utils import run_bass_kernel_spmd
from concourse.masks import make_identity

# problem shapes (hardcoded per contract)
B, C, HH, WW = 32, 512, 14, 14
HW = HH * WW            # 196 pixels
D = 8192                # sketch/output dim
NF = D // 2 + 1         # 4097 rfft bins
NMT = 33                # freq tiles of 128 (33*128 = 4224 >= 4097)
FPAD = NMT * 128        # 4224
NCORES = 8
NB = B // NCORES        # 4 batch images per core
KC = C // 128           # 4 contraction chunks
P = 128
F32 = mybir.dt.float32
BF16 = mybir.dt.bfloat16
BF16_NP = ml_dtypes.bfloat16

TRACE = False           # set by test harness for profiling runs
LAST_RESULTS = None     # BassKernelResults of the last run (for the harness)

_CACHE = {}


# ---------------------------------------------------------------- host consts
def _extract_sketch(S):
    """S is [C, D] with one nonzero (+-1) per row -> (h, s)."""
    S = np.asarray(S, dtype=np.float32)
    h = np.argmax(np.abs(S), axis=1)
    s = S[np.arange(S.shape[0]), h]
    return h, s


def _make_F(S, half_edges):
    """F[c, k] = s[c]*exp(-2i pi h[c] k / D), k in [0, FPAD); pad bins zeroed.
    half_edges also folds the irfft 1/2 weight of the DC/Nyquist bins in."""
    h, s = _extract_sketch(S)
    k = np.arange(FPAD)
    ang = (2.0 * np.pi / D) * np.outer(h.astype(np.float64), k)
    Fr = s[:, None] * np.cos(ang)
    Fi = -s[:, None] * np.sin(ang)
    Fr[:, NF:] = 0.0
    Fi[:, NF:] = 0.0
    if half_edges:
        Fr[:, 0] *= 0.5
        Fr[:, D // 2] *= 0.5
        Fi[:, 0] *= 0.5
        Fi[:, D // 2] *= 0.5
    # layout [kc, 128, FPAD] bf16 for direct DMA into lhsT tiles
    Fr = Fr.reshape(KC, 128, FPAD).astype(BF16_NP)
    Fi = Fi.reshape(KC, 128, FPAD).astype(BF16_NP)
    return Fr, Fi


def _ifft_consts():
    """irfft(Spec)[64*j1 + j2] = 2/D * Re( sum_k1 W[k1,j1] T[k1,j2]
                                   * sum_k2 Spec[k1 + 128*k2] E[k2,j2] )."""
    j2 = np.arange(64)[None, :]
    k2 = np.arange(NMT)[:, None]
    angE = 2.0 * np.pi * k2 * j2 / 64.0
    e32r = ((2.0 / D) * np.cos(angE)).astype(np.float32)
    e32i = ((2.0 / D) * np.sin(angE)).astype(np.float32)
    k1 = np.arange(128)[:, None]
    angT = 2.0 * np.pi * k1 * j2 / D
    twr = np.cos(angT).astype(np.float32)
    twi = np.sin(angT).astype(np.float32)
    j1 = np.arange(128)[None, :]
    angW = 2.0 * np.pi * np.arange(128)[:, None] * j1 / 128.0
    w128r = np.cos(angW).astype(np.float32)
    w128ni = (-np.sin(angW)).astype(np.float32)
    return {"e32r": e32r, "e32i": e32i, "e32ni": (-e32i).copy(),
            "twr": twr, "twi": twi, "w128r": w128r, "w128ni": w128ni}


def _shard_bottom(bottom):
    """[B, C, 14, 14] f32 -> per-core [KC, 128, NB, 196] bf16."""
    a = np.asarray(bottom, dtype=np.float32).reshape(NCORES, NB, KC, 128, HW)
    a = a.transpose(0, 2, 3, 1, 4)          # [core, kc, 128, b, 196]
    a = np.ascontiguousarray(a).astype(BF16_NP)
    return [np.ascontiguousarray(a[i]) for i in range(NCORES)]


# ---------------------------------------------------------------- bass program
def _build_nc():
    nc = bacc.Bacc("TRN2", target_bir_lowering=False, num_devices=NCORES)

    b1_d = nc.dram_tensor("b1", [KC, P, NB, HW], BF16, kind="ExternalInput")
    b2_d = nc.dram_tensor("b2", [KC, P, NB, HW], BF16, kind="ExternalInput")
    f_d = {w: nc.dram_tensor(w, [KC, P, FPAD], BF16, kind="ExternalInput")
           for w in ("f1r", "f1i", "f2r", "f2i")}
    cn_d = {}
    for nm, shp in (("e32r", [NMT, 64]), ("e32i", [NMT, 64]), ("e32ni", [NMT, 64]),
                    ("twr", [P, 64]), ("twi", [P, 64]),
                    ("w128r", [P, P]), ("w128ni", [P, P])):
        cn_d[nm] = nc.dram_tensor(nm, shp, F32, kind="ExternalInput")
    out_d = nc.dram_tensor("out", [NB, D], F32, kind="ExternalOutput")
    out_v = out_d.ap().rearrange("b (p f) -> b p f", p=P)

    mult = mybir.AluOpType.mult
    add = mybir.AluOpType.add

    with tile.TileContext(nc) as tc:
        with (
            tc.tile_pool(name="consts", bufs=1) as consts,
            tc.tile_pool(name="spec", bufs=1) as specp,
            tc.tile_pool(name="scratch", bufs=3) as scratch,
            tc.tile_pool(name="tmps", bufs=3) as tmps,
        ):
            # --- small constants
            identity = consts.tile([P, P], F32, name="identity", tag="identity")
            make_identity(nc, identity)
            cn = {}
            for nm in cn_d:
                t = consts.tile(list(cn_d[nm].shape), F32, name=f"c_{nm}", tag=f"c_{nm}")
                nc.sync.dma_start(out=t, in_=cn_d[nm].ap())
                cn[nm] = t

            # --- bottom activations, resident whole kernel
            bt1, bt2 = [], []
            for kc in range(KC):
                t1 = consts.tile([P, NB, HW], BF16, name=f"bt1_{kc}", tag=f"bt1_{kc}")
                nc.sync.dma_start(out=t1, in_=b1_d.ap()[kc])
                bt1.append(t1)
                t2 = consts.tile([P, NB, HW], BF16, name=f"bt2_{kc}", tag=f"bt2_{kc}")
                nc.sync.dma_start(out=t2, in_=b2_d.ap()[kc])
                bt2.append(t2)

            # --- DFT-of-sketch weights, resident; DMA'd in freq-major chunks
            # so early freq tiles are ready quickly.
            fw = {w: [consts.tile([P, FPAD], BF16, name=f"{w}_{kc}", tag=f"{w}_{kc}")
                      for kc in range(KC)] for w in f_d}
            for g0 in range(0, FPAD, 512):
                g1 = min(g0 + 512, FPAD)
                for w in fw:
                    for kc in range(KC):
                        nc.sync.dma_start(out=fw[w][kc][:, g0:g1],
                                          in_=f_d[w].ap()[kc, :, g0:g1])

            # --- spectra accumulators [128 = k mod 128, NMT = k div 128]
            specR = [specp.tile([P, NMT], F32, name=f"specR{b}", tag=f"specR{b}")
                     for b in range(NB)]
            specI = [specp.tile([P, NMT], F32, name=f"specI{b}", tag=f"specI{b}")
                     for b in range(NB)]

            # ---------------- stage 1+2: spectra of both inputs, multiplied
            # and pixel-reduced, one (freq-tile, batch) unit at a time.
            with tc.tile_pool(name="ps1", bufs=4, space="PSUM") as ps1:
                for mt in range(NMT):
                    c0 = mt * P
                    for b in range(NB):
                        # [128, 2, 196]: region 0 = real part, 1 = imag part.
                        # Sequential accumulation groups within one bank.
                        p1 = ps1.tile([P, 2, HW], F32, name=f"p1_{mt}_{b}", tag="p1")
                        p2 = ps1.tile([P, 2, HW], F32, name=f"p2_{mt}_{b}", tag="p2")
                        r1 = [bt1[kc][:, b, :] for kc in range(KC)]
                        r2 = [bt2[kc][:, b, :] for kc in range(KC)]
                        for reg, wn in ((0, "f1r"), (1, "f1i")):
                            for kc in range(KC):
                                nc.tensor.matmul(p1[:, reg, :],
                                                 fw[wn][kc][:, c0:c0 + P], r1[kc],
                                                 start=kc == 0, stop=kc == KC - 1)
                        for reg, wn in ((0, "f2r"), (1, "f2i")):
                            for kc in range(KC):
                                nc.tensor.matmul(p2[:, reg, :],
                                                 fw[wn][kc][:, c0:c0 + P], r2[kc],
                                                 start=kc == 0, stop=kc == KC - 1)
                        # Stage fft2 through SBUF (one PSUM operand per DVE op),
                        # folding the complex-product signs/swaps into the copies:
                        #   sbR = [f2r | -f2i],  sbI = [f2i | f2r]
                        sbR = scratch.tile([P, 2, HW], F32, name=f"sbR_{mt}_{b}",
                                           tag="sbR")
                        sbI = scratch.tile([P, 2, HW], F32, name=f"sbI_{mt}_{b}",
                                           tag="sbI")
                        nc.scalar.copy(sbR[:, 0, :], p2[:, 0, :])
                        nc.scalar.mul(sbR[:, 1, :], p2[:, 1, :], -1.0)
                        nc.scalar.copy(sbI[:, 0, :], p2[:, 1, :])
                        nc.scalar.copy(sbI[:, 1, :], p2[:, 0, :])
                        dst = scratch.tile([P, 2, HW], F32, name=f"dst_{mt}_{b}",
                                           tag="dst")
                        dst2 = scratch.tile([P, 2, HW], F32, name=f"dst2_{mt}_{b}",
                                            tag="dst2")
                        # SpecR = sum(r1*r2 - i1*i2);  SpecI = sum(r1*i2 + i1*r2)
                        nc.vector.scalar_tensor_tensor(
                            out=dst, in0=p1, scalar=1.0, in1=sbR,
                            op0=mult, op1=mult,
                            accum_out=specR[b][:, mt:mt + 1])
                        nc.vector.scalar_tensor_tensor(
                            out=dst2, in0=p1, scalar=1.0, in1=sbI,
                            op0=mult, op1=mult,
                            accum_out=specI[b][:, mt:mt + 1])

            # ---------------- stage 3+4: per image, transpose spectrum and run
            # the 128x64 Cooley-Tukey irfft as small fp32 matmuls.
            with tc.tile_pool(name="ps2", bufs=2, space="PSUM") as ps2:
                for b in range(NB):
                    ptr = ps2.tile([NMT, P], F32, name=f"ptr_{b}", tag="tr")
                    nc.tensor.transpose(ptr, specR[b], identity)
                    s2r = scratch.tile([NMT, P], F32, name=f"s2r_{b}", tag="s2r")
                    nc.scalar.copy(s2r, ptr)
                    pti = ps2.tile([NMT, P], F32, name=f"pti_{b}", tag="tr")
                    nc.tensor.transpose(pti, specI[b], identity)
                    s2i = scratch.tile([NMT, P], F32, name=f"s2i_{b}", tag="s2i")
                    nc.scalar.copy(s2i, pti)

                    ur = ps2.tile([P, 64], F32, name=f"ur_{b}", tag="ur")
                    ui = ps2.tile([P, 64], F32, name=f"ui_{b}", tag="ui")
                    nc.tensor.matmul(ur, s2r, cn["e32r"], start=True, stop=False)
                    nc.tensor.matmul(ur, s2i, cn["e32ni"], start=False, stop=True)
                    nc.tensor.matmul(ui, s2r, cn["e32i"], start=True, stop=False)
                    nc.tensor.matmul(ui, s2i, cn["e32r"], start=False, stop=True)

                    vr = scratch.tile([P, 64], F32, name=f"vr_{b}", tag="vr")
                    vi = scratch.tile([P, 64], F32, name=f"vi_{b}", tag="vi")
                    ta = scratch.tile([P, 64], F32, name=f"ta_{b}", tag="ta")
                    tb = scratch.tile([P, 64], F32, name=f"tb_{b}", tag="tb")
                    nc.vector.tensor_mul(vr, ur, cn["twr"])
                    nc.vector.tensor_mul(ta, ui, cn["twi"])
                    nc.vector.tensor_sub(vr, vr, ta)
                    nc.vector.tensor_mul(vi, ur, cn["twi"])
                    nc.vector.tensor_mul(tb, ui, cn["twr"])
                    nc.vector.tensor_add(vi, vi, tb)

                    px = ps2.tile([P, 64], F32, name=f"px_{b}", tag="px")
                    nc.tensor.matmul(px, cn["w128r"], vr, start=True, stop=False)
                    nc.tensor.matmul(px, cn["w128ni"], vi, start=False, stop=True)
                    xo = scratch.tile([P, 64], F32, name=f"xo_{b}", tag="xo")
                    nc.vector.tensor_copy(out=xo, in_=px)
                    nc.sync.dma_start(out=out_v[b], in_=xo)

    nc.compile()
    return nc


# ---------------------------------------------------------------- entry point
def kernel(bottom1, bottom2, S1, S2):
    global LAST_RESULTS
    bottom1 = np.asarray(bottom1, dtype=np.float32)
    bottom2 = np.asarray(bottom2, dtype=np.float32)

    if "nc" not in _CACHE:
        _CACHE["nc"] = _build_nc()
    nc = _CACHE["nc"]

    f1r, f1i = _make_F(S1, half_edges=True)
    f2r, f2i = _make_F(S2, half_edges=False)
    cns = _ifft_consts()
    shared = {"f1r": f1r, "f1i": f1i, "f2r": f2r, "f2i": f2i, **cns}

    b1s = _shard_bottom(bottom1)
    b2s = _shard_bottom(bottom2)
    in_maps = [{"b1": b1s[i], "b2": b2s[i], **shared} for i in range(NCORES)]

    res = run_bass_kernel_spmd(nc, in_maps, core_ids=list(range(NCORES)),
                               trace=TRACE)
    LAST_RESULTS = res
    out = np.concatenate([r["out"] for r in res.results], axis=0)
    return out.astype(np.float32)
